# revision 1
# baseline (speedup 1.0000x reference)
"""Cross-attention kernel for 8 Trainium2 NeuronCores.

Problem (hardcoded): x [4,4096,512], context [4,1024,768], 8 heads x 64,
inner 512. out = softmax((x@Wq)(ctx@Wk)^T / 8) @ (ctx@Wv) @ Wo + bo.

Sharding: 8 cores = 4 batches x 2 head-groups (4 heads each).
Core c handles batch b=c//2, heads [4g, 4g+4) with g=c%2:
  - Wq/Wk/Wv column-sliced, Wo row-sliced (tensor parallel over heads)
  - each core emits a partial [4096, 512]; host sums the two head-group
    partials per batch and adds bo (the only host math; everything else
    runs on device).

Device-side layout choices:
  - host pre-transposes x/context so all projections contract naturally
    (feature dim on partitions); no on-device transposes at all.
  - qT/kT hold head pairs stacked on partitions (rows 0-63 = even head,
    64-127 = odd head) so the K=64 score matmuls for the two heads occupy
    disjoint PE row-groups and run concurrently (full array).
  - scores are built transposed [keys, q] so softmax exp feeds the AV
    matmul directly as the moving operand; v carries an extra ones column
    so the softmax denominator falls out of the AV matmul for free.
  - exp runs in [128, 2048] ACT calls out of a 4-bank PSUM tile.
"""

import os
import sys

for _p in ("/opt/trn_rl_repo", "/root/.axon_site/_ro/trn_rl_repo"):
    if os.path.isdir(_p) and _p not in sys.path:
        sys.path.append(_p)

import ml_dtypes
import numpy as np

BF16_NP = np.float16

import concourse.bass as bass  # noqa: E402
import concourse.mybir as mybir  # noqa: E402
import concourse.tile as tile  # noqa: E402
from concourse import bacc  # noqa: E402
from concourse import bass_utils  # noqa: E402

P = 128
B = 4
NQ = 4096  # queries per batch
DX = 512  # x feature dim (4 chunks of 128)
NC = 1024  # context length (8 key chunks of 128)
DC = 768  # context feature dim (6 chunks of 128)
DH = 64  # head dim
HPC = 4  # heads per core
COLS = HPC * DH  # 256 = per-core slice of the inner dim
DOUT = 512  # output dim

DXC = DX // P  # 4
DCC = DC // P  # 6
KC = NC // P  # 8
NQT = NQ // 512  # 8 query tiles of 512

F32 = mybir.dt.float32
F32R = mybir.dt.float32r
BF16 = mybir.dt.float16
EXP = mybir.ActivationFunctionType.Exp
SCALE = DH**-0.5  # 0.125, folded into the exp activation's scale


def _r(ap):
    return ap.bitcast(F32R)


def _emit(tc, nc, xT, ctxT, wq, wk, wv, wo, ones, out):
    with (
        tc.tile_pool(name="consts", bufs=1) as consts,
        tc.tile_pool(name="xstream", bufs=3) as xstream,
        tc.tile_pool(name="etile", bufs=4) as etile,
        tc.tile_pool(name="norm", bufs=2) as norm,
        tc.tile_pool(name="dscr", bufs=4, space="DRAM") as dscr,
    ):
        # ---- weights + context into SBUF (feature dim on partitions) ----
        wq_sb = consts.tile([P, DXC, COLS], BF16, tag="wq", name="wq_sb")
        wk_sb = consts.tile([P, DCC, COLS], BF16, tag="wk", name="wk_sb")
        wv_sb = consts.tile([P, DCC, COLS], BF16, tag="wv", name="wv_sb")
        wo_sb = consts.tile([P, 2, DOUT], BF16, tag="wo", name="wo_sb")
        ctx_pool_cm = tc.tile_pool(name="ctxpool", bufs=1)
        ctx_pool = ctx_pool_cm.__enter__()
        ctxT_sb = ctx_pool.tile([P, DCC, NC], BF16, tag="ctxT", name="ctxT_sb")
        nc.sync.dma_start(wq_sb[:], wq.rearrange("(c p) n -> p c n", p=P))
        nc.sync.dma_start(wk_sb[:], wk.rearrange("(c p) n -> p c n", p=P))
        nc.sync.dma_start(wv_sb[:], wv.rearrange("(c p) n -> p c n", p=P))
        nc.sync.dma_start(wo_sb[:], wo.rearrange("(c p) n -> p c n", p=P))
        nc.sync.dma_start(ctxT_sb[:], ctxT.rearrange("(c p) n -> p c n", p=P))

        ps_scores_cm = tc.tile_pool(name="ps_scores", bufs=2, space="PSUM")
        ps_scores = ps_scores_cm.__enter__()
        ps_attn_cm = tc.tile_pool(name="ps_attn", bufs=4, space="PSUM")
        ps_attn = ps_attn_cm.__enter__()
        ps_out = ps_scores

        # ---- K^T projection: kT[pair][2*64 head dims, 1024 keys] ----
        kT_sb = [consts.tile([P, NC], BF16, tag=f"kT{p}", name=f"kT{p}") for p in range(2)]
        for p in range(2):
            for ks in range(2):
                acc = ps_scores.tile([P, 2, 512], F32, tag="scores", name="kproj_acc")[:, 0, :]
                for ch in range(DCC):
                    nc.tensor.matmul(
                        acc[:],
                        wk_sb[:, ch, p * P : (p + 1) * P],
                        ctxT_sb[:, ch, ks * 512 : (ks + 1) * 512],
                        start=(ch == 0),
                        stop=(ch == DCC - 1),
                    )
                nc.vector.tensor_copy(kT_sb[p][:, ks * 512 : (ks + 1) * 512], acc[:])

        # ---- V projection, keys on partitions, + ones column ----
        # v_sb[:, kc, h, 0:64] = V for head h, key chunk kc; [..., 64] = 1.0
        v_sb = consts.tile([P, KC, HPC, DH + 1], BF16, tag="v", name="v_sb")
        # memset cannot write f32r; broadcast a DRAM 1.0 into the ones column
        nc.sync.dma_start(
            v_sb[:, :, :, DH : DH + 1].rearrange("p a b o -> p (a b o)"),
            ones.to_broadcast((P, KC * HPC)),
        )
        for kc in range(KC):
            acc = ps_scores.tile([P, 2, 512], F32, tag="scores", name="vproj_acc")[:, 0, :]
            for ch in range(DCC):
                nc.tensor.matmul(
                    acc[:, 0:COLS],
                    ctxT_sb[:, ch, kc * P : (kc + 1) * P],
                    wv_sb[:, ch, :],
                    start=(ch == 0),
                    stop=(ch == DCC - 1),
                )
            nc.vector.tensor_copy(
                v_sb[:, kc, :, 0:DH], acc[:, 0:COLS].rearrange("p (h d) -> p h d", d=DH)
            )

        # ---- Q^T projection, emitted lazily two tiles ahead of use ----
        qT_sb = {}

        def emit_qproj(qs):
            xt = xstream.tile([P, DXC, 512], BF16, tag="xt", name="xt")
            nc.sync.dma_start(
                xt[:],
                xT.rearrange("(c p) q -> p c q", p=P)[:, :, qs * 512 : (qs + 1) * 512],
            )
            for p in range(2):
                acc = ps_scores.tile([P, 2, 512], F32, tag="scores", name="qproj_acc")[:, 0, :]
                for ch in range(DXC):
                    nc.tensor.matmul(
                        acc[:],
                        wq_sb[:, ch, p * P : (p + 1) * P],
                        xt[:, ch, :],
                        start=(ch == 0),
                        stop=(ch == DXC - 1),
                    )
                qt_t = consts.tile([P, 512], BF16, tag=f"qT{p}_{qs}", name=f"qT{p}_{qs}")
                qT_sb[(p, qs)] = qt_t
                nc.vector.tensor_copy(qt_t[:], acc[:])

        emit_qproj(0)
        emit_qproj(1)

        ctx_pool_cm.__exit__(None, None, None)

        # ---- attention + output projection, per 512-query tile ----
        # outproj for qt is emitted after attention(qt+1) so the PE never
        # waits out the normalize chain at a qt boundary
        attnT_all = {}

        def outproj(qt):
            for sub in range(4):
                o = ps_out.tile([P, 2, 512], F32, tag="scores", name="oproj_acc")[:, 0, :]
                for p in range(2):
                    nc.tensor.matmul(
                        o[:],
                        attnT_all[(p, qt)][:, sub * P : (sub + 1) * P],
                        wo_sb[:, p, :],
                        start=(p == 0),
                        stop=(p == 1),
                    )
                ostage = norm.tile([P, DOUT], F32, tag="ostage", name="ostage_t")
                nc.vector.tensor_copy(ostage[:], o[:])
                row = qt * 512 + sub * P
                nc.gpsimd.dma_start(out[row : row + P, :], ostage[:])

        def attn_compute(qt, p):
            qt_t = qT_sb[(p, qt)]
            accs = [ps_attn.tile([DH + 1, 512], F32, tag="attnT", name="attn_acc") for _ in range(2)]
            for kc in range(KC):
                sc = ps_scores.tile([P, 2, 512], F32, tag="scores", name="scores_ps")
                for j in range(2):
                    nc.tensor.matmul(
                        sc[:, j, :],
                        kT_sb[p][j * DH : (j + 1) * DH, kc * P : (kc + 1) * P],
                        qt_t[j * DH : (j + 1) * DH, :],
                        start=True,
                        stop=True,
                    )
                ex = etile.tile([P, 2, 512], BF16, tag="exp", name="exp_sb")
                nc.scalar.activation(ex[:], sc[:], EXP, scale=SCALE)
                for j in range(2):
                    nc.tensor.matmul(
                        accs[j][:],
                        v_sb[:, kc, 2 * p + j, :],
                        ex[:, j, :],
                        start=(kc == 0),
                        stop=(kc == KC - 1),
                    )
            return accs

        def attn_normalize(qt, p, accs):
            at_t = consts.tile([P, 512], BF16, tag=f"attnT{p}_{qt}", name=f"attnT{p}_{qt}")
            attnT_all[(p, qt)] = at_t
            dstage = norm.tile([DH + 1, 2, 512], F32, tag="denom", name="den_t")
            for j in range(2):
                nc.vector.tensor_copy(
                    dstage[DH : DH + 1, j, :], accs[j][DH : DH + 1, :]
                )
            dden = dscr.tile([1, 1024], F32, tag="dden", name="dden_t")
            nc.gpsimd.dma_start(dden[:], dstage[DH : DH + 1, :, :])
            rt = norm.tile([P, 8], F32, tag="rt", name="rt_t")
            nc.gpsimd.dma_start(rt[:], dden[0, :].rearrange("(p f) -> p f", p=P))
            nc.vector.reciprocal(rt[:], rt[:])
            drec = dscr.tile([1, 1024], F32, tag="drec", name="drec_t")
            nc.gpsimd.dma_start(drec[:], rt[:])
            for j in range(2):
                rec = norm.tile([DH, 512], F32, tag="recip", name="recip_t")
                nc.gpsimd.dma_start(
                    rec[:],
                    drec[:, j * 512 : (j + 1) * 512].to_broadcast((DH, 512)),
                )
                if j == 0:
                    nc.vector.tensor_mul(at_t[0:DH, :], accs[j][0:DH, :], rec[:])
                else:
                    tmp = norm.tile([DH, 512], BF16, tag="normtmp", name="normtmp_t")
                    nc.vector.tensor_mul(tmp[:], accs[j][0:DH, :], rec[:])
                    # engines cannot shift partitions; DMA moves the odd
                    # head's rows into partitions 64-127
                    nc.gpsimd.dma_start(at_t[DH:P, :], tmp[:])

        for qt in range(NQT):
            accs0 = attn_compute(qt, 0)
            accs1 = attn_compute(qt, 1)
            attn_normalize(qt, 0, accs0)
            attn_normalize(qt, 1, accs1)
            if qt + 2 < NQT:
                emit_qproj(qt + 2)
            if qt >= 1:
                outproj(qt - 1)
        outproj(NQT - 1)
        ps_attn_cm.__exit__(None, None, None)
        ps_scores_cm.__exit__(None, None, None)


def _build():
    nc = bacc.Bacc(
        "TRN2", target_bir_lowering=False, debug=False, enable_asserts=False
    )
    xT = nc.dram_tensor("xT", [DX, NQ], BF16, kind="ExternalInput").ap()
    ctxT = nc.dram_tensor("ctxT", [DC, NC], BF16, kind="ExternalInput").ap()
    wq = nc.dram_tensor("wq", [DX, COLS], BF16, kind="ExternalInput").ap()
    wk = nc.dram_tensor("wk", [DC, COLS], BF16, kind="ExternalInput").ap()
    wv = nc.dram_tensor("wv", [DC, COLS], BF16, kind="ExternalInput").ap()
    wo = nc.dram_tensor("wo", [COLS, DOUT], BF16, kind="ExternalInput").ap()
    ones = nc.dram_tensor("ones", [1, KC * HPC], BF16, kind="ExternalInput").ap()
    out = nc.dram_tensor("out", [NQ, DOUT], F32, kind="ExternalOutput").ap()
    with tile.TileContext(nc) as tc:
        _emit(tc, nc, xT, ctxT, wq, wk, wv, wo, ones, out)
    nc.compile()
    return nc


_NC = None


def _get_nc():
    global _NC
    if _NC is None:
        _NC = _build()
    return _NC


def _in_maps(x, context, Wq, Wk, Wv, Wo):
    maps = []
    for c in range(8):
        b, g = c // 2, c % 2
        cs = slice(g * COLS, (g + 1) * COLS)
        maps.append(
            {
                "xT": np.ascontiguousarray(x[b].T.astype(BF16_NP)),
                "ctxT": np.ascontiguousarray(context[b].T.astype(BF16_NP)),
                "wq": np.ascontiguousarray(Wq[:, cs].astype(BF16_NP)),
                "wk": np.ascontiguousarray(Wk[:, cs].astype(BF16_NP)),
                "wv": np.ascontiguousarray(Wv[:, cs].astype(BF16_NP)),
                "wo": np.ascontiguousarray(Wo[cs, :].astype(BF16_NP)),
                "ones": np.ones((1, KC * HPC), BF16_NP),
            }
        )
    return maps


def _execute(in_maps, **kw):
    return bass_utils.run_bass_kernel_spmd(
        _get_nc(), in_maps, core_ids=list(range(8)), **kw
    )


def kernel(x, context, Wq, Wk, Wv, Wo, bo):
    x = np.asarray(x, np.float32)
    context = np.asarray(context, np.float32)
    Wq = np.asarray(Wq, np.float32)
    Wk = np.asarray(Wk, np.float32)
    Wv = np.asarray(Wv, np.float32)
    Wo = np.asarray(Wo, np.float32)
    bo = np.asarray(bo, np.float32)
    res = _execute(_in_maps(x, context, Wq, Wk, Wv, Wo))
    parts = [r["out"] for r in res.results]
    out = np.empty((B, NQ, DOUT), np.float32)
    for b in range(B):
        out[b] = parts[2 * b] + parts[2 * b + 1] + bo[None, :]
    return out

